# revision 29
# baseline (speedup 1.0000x reference)
"""Trainium2 Bass kernel for a feature-space attention head.

Reference computation (per batch b, with T=4096, E=1024, D=64):
    Q = x @ Wq; K = x @ Wk; V = x @ Wv            # (T,E)@(E,D) -> (T,D)
    R = (K^T @ Q) / sqrt(E)                        # (D,D) feature-space scores
    R = where(strictly_lower, -inf, R); R = softmax(R, axis=-1)
    out = V @ R                                    # (T,D)

Sharding: data-parallel over batch B=8 across the 8 NeuronCores (one batch
per core, no collectives).

Per-core device pipeline (bf16 operands, fp32 PSUM accumulation):
  - host pre-casts to bf16 and uploads x TRANSPOSED (x^T [E, T]) so the
    device never transposes x; W is packed partition-major as
    w3 = [Wq/32 | Wk | Wv] (score scale folded into Wq); PE-transpose
    identity + softmax mask are uploaded as constants.
  - DMA engines serve all queues round-robin with no priority, so w3 and
    ALL x pieces go on ONE queue (sync) in exact consumption order:
    per-queue FIFO makes completion order = need order. Fine 2-tile pieces
    first (PE starts ~11us), coarse 4-tile pieces mid-stream, fine tail.
  - per 128-row t-tile: one joint QKV matmul pass (stationary = x^T chunk,
    moving = w3 [128,192]) -> QKV natural in PSUM; ACT/DVE copy to bf16.
  - per 8-tile batch (PSUM bank switches cost ~120ns): 8x R += K^T Q into
    a persistent PSUM bank, then 8 V-tile PE transposes into one [64,1024]
    PSUM tile, one DVE copy into persistent V^T. The final batch is split
    4+4 so the post-QKV serial tail is short.
  - softmax on R (64x64) in fp32 without max-shift (scores bounded ~|12|,
    exp is f32-safe; masked entries underflow to 0), O = V @ P via
    V^T-stationary chunks, 8 output groups alternating both HWDGE queues.
"""

import os
import sys

import numpy as np

for _p in ("/opt/trn_rl_repo", "/root/.axon_site/_ro/trn_rl_repo"):
    if os.path.isdir(_p) and _p not in sys.path:
        sys.path.append(_p)

import ml_dtypes  # noqa: E402

import concourse.bass as bass  # noqa: E402
import concourse.tile as tile  # noqa: E402
from concourse import bacc, mybir  # noqa: E402
from concourse.bass_utils import run_bass_kernel_spmd  # noqa: E402

B, T, E, D = 8, 4096, 1024, 64
N_CORES = 8
M3 = 3 * D                # 192: packed [Q|K|V] output columns
ECH = E // 128            # 8 e-chunks
NT = T // 128             # 32 t-tiles

F32 = mybir.dt.float32
BF16 = mybir.dt.bfloat16
AX = mybir.AxisListType
AF = mybir.ActivationFunctionType

_COMPILED = None

# x-piece schedule (start tile, #tiles): fine first for early PE start,
# coarse mid-stream, fine tail so the last QKV isn't gated on a big piece
X_PIECES = [(0, 1), (1, 1), (2, 2), (4, 2), (6, 2),
            (8, 4), (12, 4), (16, 4), (20, 4), (24, 4),
            (28, 2), (30, 1), (31, 1)]
# R/Vt emission batches (start, len): last batch small for a short tail
R_BATCHES = [(0, 8), (8, 8), (16, 8), (24, 4), (28, 4)]


def _build():
    nc = bacc.Bacc("TRN2", target_bir_lowering=False, debug=False,
                   num_devices=N_CORES)
    xt = nc.dram_tensor("xt", [E, T], BF16, kind="ExternalInput").ap()
    # w3 pre-packed partition-major on host: [128, ECH*M3]
    w3 = nc.dram_tensor("w3", [128, ECH * M3], BF16,
                        kind="ExternalInput").ap()
    ident = nc.dram_tensor("ident", [128, 128], BF16,
                           kind="ExternalInput").ap()
    mask = nc.dram_tensor("mask", [64, 64], F32, kind="ExternalInput").ap()
    # p-major output: row p holds O[n*128+p, :] for n=0..31 contiguously
    # (1KB descriptors instead of 256B; host un-permutes)
    out = nc.dram_tensor("out", [128, NT * D], F32, kind="ExternalOutput").ap()

    xt_r = xt.rearrange("(c p) t -> p c t", p=128)        # [128, 8, 4096]

    with tile.TileContext(nc) as tc:
        with (
            tc.tile_pool(name="const", bufs=1) as constp,
            tc.tile_pool(name="xs", bufs=1) as xsp,
            tc.tile_pool(name="qkv", bufs=12) as qkvp,
            tc.tile_pool(name="vt", bufs=1) as vtp,
            tc.tile_pool(name="small", bufs=1) as smallp,
            tc.tile_pool(name="osb", bufs=8) as osbp,
            tc.tile_pool(name="ps_qkv", bufs=3, space="PSUM") as ps_qkv,
            tc.tile_pool(name="ps_vt", bufs=1, space="PSUM") as ps_vt,
            tc.tile_pool(name="ps_r", bufs=1, space="PSUM") as ps_rp,
            tc.tile_pool(name="ps_o", bufs=3, space="PSUM") as ps_o,
        ):
            # single ordered stream on sync: w3 first, then x pieces in
            # consumption order
            w3_sb = constp.tile([128, ECH * M3], BF16)
            nc.sync.dma_start(w3_sb[:], w3[:])

            xt_of = [None] * NT  # tile i -> (tile_ap, pitch, col0)
            for t0, ntl in X_PIECES:
                xs = xsp.tile([128, ECH * ntl * 128], BF16,
                              tag=f"xs{t0}", name="xs")
                nc.sync.dma_start(
                    xs[:].rearrange("p (c t) -> p c t", c=ECH),
                    xt_r[:, :, t0 * 128:(t0 + ntl) * 128])
                for s in range(ntl):
                    xt_of[t0 + s] = (xs, ntl * 128, s * 128)

            # constants on gpsimd (identity needed by the first Vt flush)
            ident_sb = constp.tile([128, 128], BF16)
            nc.gpsimd.dma_start(ident_sb[:], ident[:])
            mask_sb = constp.tile([64, 64], F32)
            nc.gpsimd.dma_start(mask_sb[:], mask[:])

            vT = vtp.tile([64, T], BF16)          # persistent V^T
            ps_R = ps_rp.tile([64, 64], F32)      # persistent R accumulator

            qkv_tiles = [None] * NT
            flush_at = {}  # qkv-tile index -> batch to emit when reached
            for bi, (b0, blen) in enumerate(R_BATCHES):
                nxt = b0 + blen  # first tile whose QKV burst hosts the flush
                flush_at[min(nxt + 1, NT - 1) if nxt < NT else NT] = (b0, blen)

            def emit_batch(b0, blen):
                for i in range(b0, b0 + blen):
                    nc.tensor.matmul(
                        ps_R[:], qkv_tiles[i][:, D:2 * D],
                        qkv_tiles[i][:, 0:D],
                        start=(i == 0), stop=(i == NT - 1),
                    )
                pvt = ps_vt.tile([64, blen * 128], BF16, tag="vt")
                for n in range(blen):
                    nc.tensor.transpose(
                        pvt[:, n * 128:(n + 1) * 128],
                        qkv_tiles[b0 + n][:, 2 * D:3 * D], ident_sb[:],
                    )
                nc.vector.tensor_copy(
                    vT[:, b0 * 128:(b0 + blen) * 128], pvt[:])

            for i in range(NT):
                xs, pitch, c0 = xt_of[i]
                pq = ps_qkv.tile([128, M3], F32, tag="qkv")
                for j in range(ECH):
                    nc.tensor.matmul(
                        pq[:], xs[:, j * pitch + c0:j * pitch + c0 + 128],
                        w3_sb[:, j * M3:(j + 1) * M3],
                        start=(j == 0), stop=(j == ECH - 1),
                    )
                    if j == 2 and i in flush_at:
                        emit_batch(*flush_at.pop(i))
                qkv_sb = qkvp.tile([128, M3], BF16, tag="qkv_sb")
                if i % 2 == 0:
                    nc.scalar.activation(qkv_sb[:], pq[:], AF.Copy)
                else:
                    nc.vector.tensor_copy(qkv_sb[:], pq[:])
                qkv_tiles[i] = qkv_sb
            if NT in flush_at:
                emit_batch(*flush_at.pop(NT))

            # ---- softmax on R (64x64), no max-shift (bounded scores) ----
            r_sb = smallp.tile([64, 64], F32)
            nc.vector.tensor_add(r_sb[:], ps_R[:], mask_sb[:])
            p_exp = smallp.tile([64, 64], F32)
            rowsum = smallp.tile([64, 1], F32)
            nc.scalar.activation(p_exp[:], r_sb[:], AF.Exp,
                                 bias=0.0, scale=1.0, accum_out=rowsum[:])
            rinv = smallp.tile([64, 1], F32)
            nc.vector.reciprocal(rinv[:], rowsum[:])
            p_r = smallp.tile([64, 64], BF16)
            nc.vector.tensor_scalar_mul(p_r[:], p_exp[:], rinv[:])

            # ---- O = V @ P : 4 groups of 8 chunks, DMAs on both queues ----
            for grp in range(4):
                po = ps_o.tile([128, 8 * D], F32, tag="o")
                for k in range(8):
                    c = grp * 8 + k
                    nc.tensor.matmul(
                        po[:, k * D:(k + 1) * D],
                        vT[:, c * 128:(c + 1) * 128], p_r[:],
                        start=True, stop=True,
                    )
                o_sb = osbp.tile([128, 8 * D], F32, tag="o_sb")
                if grp % 2 == 0:
                    nc.scalar.activation(o_sb[:], po[:], AF.Copy)
                else:
                    nc.vector.tensor_copy(o_sb[:], po[:])
                q = nc.sync if grp % 2 == 0 else nc.scalar
                q.dma_start(out[:, grp * 8 * D:(grp + 1) * 8 * D], o_sb[:])

    nc.compile()
    return nc


def _host_inputs(x, Wq, Wk, Wv):
    """Host-side prep: bf16 casts, x transpose, weight packing, constants."""
    bf16 = ml_dtypes.bfloat16
    # fold the 1/sqrt(E) score scale into Wq (1/32 is exact in f32)
    w3f = np.concatenate(
        [np.asarray(Wq, np.float32) * (1.0 / 32.0),
         np.asarray(Wk, np.float32),
         np.asarray(Wv, np.float32)], axis=1).astype(bf16)  # [E, 192]
    # partition-major pack: w3h[p, c*192+m] = w3f[c*128+p, m]
    w3h = np.ascontiguousarray(
        w3f.reshape(ECH, 128, M3).transpose(1, 0, 2).reshape(128, ECH * M3))
    ident_h = np.eye(128, dtype=bf16)
    ii = np.arange(64)
    # additive mask: 0 where col >= row, -1e30 strictly below the diagonal
    mask_h = np.where(ii[None, :] >= ii[:, None], np.float32(0.0),
                      np.float32(-1e30)).astype(np.float32)
    xb = np.asarray(x, np.float32).astype(bf16)  # (B, T, E)
    xtb = np.ascontiguousarray(xb.transpose(0, 2, 1))  # (B, E, T)
    return [
        {"xt": xtb[b], "w3": w3h, "ident": ident_h, "mask": mask_h}
        for b in range(B)
    ]


def kernel(x, Wq, Wk, Wv):
    global _COMPILED
    if _COMPILED is None:
        _COMPILED = _build()
    nc = _COMPILED

    in_maps = _host_inputs(x, Wq, Wk, Wv)
    res = run_bass_kernel_spmd(nc, in_maps, list(range(N_CORES)))
    # un-permute p-major device layout: out2[p, n*64+d] = O[n*128+p, d]
    return np.stack([
        np.ascontiguousarray(
            res.results[b]["out"].reshape(128, NT, D)
            .transpose(1, 0, 2).reshape(T, D))
        for b in range(B)
    ], axis=0)


# revision 30
# speedup vs baseline: 1.0285x; 1.0285x over previous
"""Trainium2 Bass kernel for a feature-space attention head.

Reference computation (per batch b, with T=4096, E=1024, D=64):
    Q = x @ Wq; K = x @ Wk; V = x @ Wv            # (T,E)@(E,D) -> (T,D)
    R = (K^T @ Q) / sqrt(E)                        # (D,D) feature-space scores
    R = where(strictly_lower, -inf, R); R = softmax(R, axis=-1)
    out = V @ R                                    # (T,D)

Sharding: data-parallel over batch B=8 across the 8 NeuronCores (one batch
per core, no collectives).

Per-core device pipeline (bf16 operands, fp32 PSUM accumulation):
  - host pre-casts to bf16 and uploads x TRANSPOSED (x^T [E, T]) so the
    device never transposes x; W is packed partition-major as
    w3 = [Wq/32 | Wk | Wv] (score scale folded into Wq); PE-transpose
    identity + softmax mask are uploaded as constants.
  - DMA engines serve all queues round-robin with no priority, so w3 and
    ALL x pieces go on ONE queue (sync) in exact consumption order:
    per-queue FIFO makes completion order = need order. Fine 2-tile pieces
    first (PE starts ~11us), coarse 4-tile pieces mid-stream, fine tail.
  - per 128-row t-tile: one joint QKV matmul pass (stationary = x^T chunk,
    moving = w3 [128,192]) -> QKV natural in PSUM; ACT/DVE copy to bf16.
  - per 8-tile batch (PSUM bank switches cost ~120ns): 8x R += K^T Q into
    a persistent PSUM bank, then 8 V-tile PE transposes into one [64,1024]
    PSUM tile, one DVE copy into persistent V^T. The final batch is split
    4+4 so the post-QKV serial tail is short.
  - softmax on R (64x64) in fp32 without max-shift (scores bounded ~|12|,
    exp is f32-safe; masked entries underflow to 0), O = V @ P via
    V^T-stationary chunks, 8 output groups alternating both HWDGE queues.
"""

import os
import sys

import numpy as np

for _p in ("/opt/trn_rl_repo", "/root/.axon_site/_ro/trn_rl_repo"):
    if os.path.isdir(_p) and _p not in sys.path:
        sys.path.append(_p)

import ml_dtypes  # noqa: E402

import concourse.bass as bass  # noqa: E402
import concourse.tile as tile  # noqa: E402
from concourse import bacc, mybir  # noqa: E402
from concourse.bass_utils import run_bass_kernel_spmd  # noqa: E402

B, T, E, D = 8, 4096, 1024, 64
N_CORES = 8
M3 = 3 * D                # 192: packed [Q|K|V] output columns
ECH = E // 128            # 8 e-chunks
NT = T // 128             # 32 t-tiles

F32 = mybir.dt.float32
BF16 = mybir.dt.bfloat16
AX = mybir.AxisListType
AF = mybir.ActivationFunctionType

_COMPILED = None

# x-piece schedule (start tile, #tiles): fine first for early PE start,
# coarse mid-stream, fine tail so the last QKV isn't gated on a big piece
X_PIECES = [(0, 2), (2, 2), (4, 2), (6, 2),
            (8, 4), (12, 4), (16, 4), (20, 4), (24, 4),
            (28, 2), (30, 1), (31, 1)]
# R/Vt emission batches (start, len): last batch small for a short tail
R_BATCHES = [(0, 8), (8, 8), (16, 8), (24, 4), (28, 4)]


def _build():
    nc = bacc.Bacc("TRN2", target_bir_lowering=False, debug=False,
                   num_devices=N_CORES)
    xt = nc.dram_tensor("xt", [E, T], BF16, kind="ExternalInput").ap()
    # w3 pre-packed partition-major on host: [128, ECH*M3]
    w3 = nc.dram_tensor("w3", [128, ECH * M3], BF16,
                        kind="ExternalInput").ap()
    ident = nc.dram_tensor("ident", [128, 128], BF16,
                           kind="ExternalInput").ap()
    mask = nc.dram_tensor("mask", [64, 64], F32, kind="ExternalInput").ap()
    # p-major output: row p holds O[n*128+p, :] for n=0..31 contiguously
    # (1KB descriptors instead of 256B; host un-permutes)
    out = nc.dram_tensor("out", [128, NT * D], F32, kind="ExternalOutput").ap()

    xt_r = xt.rearrange("(c p) t -> p c t", p=128)        # [128, 8, 4096]

    with tile.TileContext(nc) as tc:
        with (
            tc.tile_pool(name="const", bufs=1) as constp,
            tc.tile_pool(name="xs", bufs=1) as xsp,
            tc.tile_pool(name="qkv", bufs=12) as qkvp,
            tc.tile_pool(name="vt", bufs=1) as vtp,
            tc.tile_pool(name="small", bufs=1) as smallp,
            tc.tile_pool(name="osb", bufs=8) as osbp,
            tc.tile_pool(name="ps_qkv", bufs=3, space="PSUM") as ps_qkv,
            tc.tile_pool(name="ps_vt", bufs=1, space="PSUM") as ps_vt,
            tc.tile_pool(name="ps_r", bufs=1, space="PSUM") as ps_rp,
            tc.tile_pool(name="ps_o", bufs=3, space="PSUM") as ps_o,
        ):
            # single ordered stream on sync: w3 first, then x pieces in
            # consumption order
            w3_sb = constp.tile([128, ECH * M3], BF16)
            nc.sync.dma_start(w3_sb[:], w3[:])

            xt_of = [None] * NT  # tile i -> (tile_ap, pitch, col0)
            for t0, ntl in X_PIECES:
                xs = xsp.tile([128, ECH * ntl * 128], BF16,
                              tag=f"xs{t0}", name="xs")
                nc.sync.dma_start(
                    xs[:].rearrange("p (c t) -> p c t", c=ECH),
                    xt_r[:, :, t0 * 128:(t0 + ntl) * 128])
                for s in range(ntl):
                    xt_of[t0 + s] = (xs, ntl * 128, s * 128)

            # constants on gpsimd (identity needed by the first Vt flush)
            ident_sb = constp.tile([128, 128], BF16)
            nc.gpsimd.dma_start(ident_sb[:], ident[:])
            mask_sb = constp.tile([64, 64], F32)
            nc.gpsimd.dma_start(mask_sb[:], mask[:])

            vT = vtp.tile([64, T], BF16)          # persistent V^T
            ps_R = ps_rp.tile([64, 64], F32)      # persistent R accumulator

            qkv_tiles = [None] * NT
            flush_at = {}  # qkv-tile index -> batch to emit when reached
            for bi, (b0, blen) in enumerate(R_BATCHES):
                nxt = b0 + blen  # first tile whose QKV burst hosts the flush
                flush_at[min(nxt + 1, NT - 1) if nxt < NT else NT] = (b0, blen)

            def emit_batch(b0, blen):
                for i in range(b0, b0 + blen):
                    nc.tensor.matmul(
                        ps_R[:], qkv_tiles[i][:, D:2 * D],
                        qkv_tiles[i][:, 0:D],
                        start=(i == 0), stop=(i == NT - 1),
                    )
                pvt = ps_vt.tile([64, blen * 128], BF16, tag="vt")
                for n in range(blen):
                    nc.tensor.transpose(
                        pvt[:, n * 128:(n + 1) * 128],
                        qkv_tiles[b0 + n][:, 2 * D:3 * D], ident_sb[:],
                    )
                nc.vector.tensor_copy(
                    vT[:, b0 * 128:(b0 + blen) * 128], pvt[:])

            for i in range(NT):
                xs, pitch, c0 = xt_of[i]
                pq = ps_qkv.tile([128, M3], F32, tag="qkv")
                for j in range(ECH):
                    nc.tensor.matmul(
                        pq[:], xs[:, j * pitch + c0:j * pitch + c0 + 128],
                        w3_sb[:, j * M3:(j + 1) * M3],
                        start=(j == 0), stop=(j == ECH - 1),
                    )
                    if j == 2 and i in flush_at:
                        emit_batch(*flush_at.pop(i))
                qkv_sb = qkvp.tile([128, M3], BF16, tag="qkv_sb")
                if i % 2 == 0:
                    nc.scalar.activation(qkv_sb[:], pq[:], AF.Copy)
                else:
                    nc.vector.tensor_copy(qkv_sb[:], pq[:])
                qkv_tiles[i] = qkv_sb
            if NT in flush_at:
                emit_batch(*flush_at.pop(NT))

            # ---- softmax on R (64x64), no max-shift (bounded scores) ----
            r_sb = smallp.tile([64, 64], F32)
            nc.vector.tensor_add(r_sb[:], ps_R[:], mask_sb[:])
            p_exp = smallp.tile([64, 64], F32)
            rowsum = smallp.tile([64, 1], F32)
            nc.scalar.activation(p_exp[:], r_sb[:], AF.Exp,
                                 bias=0.0, scale=1.0, accum_out=rowsum[:])
            rinv = smallp.tile([64, 1], F32)
            nc.vector.reciprocal(rinv[:], rowsum[:])
            p_r = smallp.tile([64, 64], BF16)
            nc.vector.tensor_scalar_mul(p_r[:], p_exp[:], rinv[:])

            # ---- O = V @ P : 4 groups of 8 chunks, DMAs on both queues ----
            for grp in range(4):
                po = ps_o.tile([128, 8 * D], F32, tag="o")
                for k in range(8):
                    c = grp * 8 + k
                    nc.tensor.matmul(
                        po[:, k * D:(k + 1) * D],
                        vT[:, c * 128:(c + 1) * 128], p_r[:],
                        start=True, stop=True,
                    )
                o_sb = osbp.tile([128, 8 * D], F32, tag="o_sb")
                if grp % 2 == 0:
                    nc.scalar.activation(o_sb[:], po[:], AF.Copy)
                else:
                    nc.vector.tensor_copy(o_sb[:], po[:])
                q = nc.sync if grp % 2 == 0 else nc.scalar
                q.dma_start(out[:, grp * 8 * D:(grp + 1) * 8 * D], o_sb[:])

    nc.compile()
    return nc


def _host_inputs(x, Wq, Wk, Wv):
    """Host-side prep: bf16 casts, x transpose, weight packing, constants."""
    bf16 = ml_dtypes.bfloat16
    # fold the 1/sqrt(E) score scale into Wq (1/32 is exact in f32)
    w3f = np.concatenate(
        [np.asarray(Wq, np.float32) * (1.0 / 32.0),
         np.asarray(Wk, np.float32),
         np.asarray(Wv, np.float32)], axis=1).astype(bf16)  # [E, 192]
    # partition-major pack: w3h[p, c*192+m] = w3f[c*128+p, m]
    w3h = np.ascontiguousarray(
        w3f.reshape(ECH, 128, M3).transpose(1, 0, 2).reshape(128, ECH * M3))
    ident_h = np.eye(128, dtype=bf16)
    ii = np.arange(64)
    # additive mask: 0 where col >= row, -1e30 strictly below the diagonal
    mask_h = np.where(ii[None, :] >= ii[:, None], np.float32(0.0),
                      np.float32(-1e30)).astype(np.float32)
    xb = np.asarray(x, np.float32).astype(bf16)  # (B, T, E)
    xtb = np.ascontiguousarray(xb.transpose(0, 2, 1))  # (B, E, T)
    return [
        {"xt": xtb[b], "w3": w3h, "ident": ident_h, "mask": mask_h}
        for b in range(B)
    ]


def kernel(x, Wq, Wk, Wv):
    global _COMPILED
    if _COMPILED is None:
        _COMPILED = _build()
    nc = _COMPILED

    in_maps = _host_inputs(x, Wq, Wk, Wv)
    res = run_bass_kernel_spmd(nc, in_maps, list(range(N_CORES)))
    # un-permute p-major device layout: out2[p, n*64+d] = O[n*128+p, d]
    return np.stack([
        np.ascontiguousarray(
            res.results[b]["out"].reshape(128, NT, D)
            .transpose(1, 0, 2).reshape(T, D))
        for b in range(B)
    ], axis=0)
